# revision 17
# baseline (speedup 1.0000x reference)
"""BANLayer Trainium2 kernel.

Data-parallel over batch: 8 examples -> 8 NeuronCores, weights replicated.

Per-core math (one example; i=num_v=128, j=num_q=512, k=in_dim=128,
hd=H_OUT*H_DIM=512, per-head d=256 split in two 128-halves; chunk index
c in 0..3 <-> (h, dhalf)):

  v_T[hd, i]  = Wv.T @ v.T + bv            (PE transpose of v, then matmul)
  qs_T[hd, j] = (Wq.T @ q.T + bq) * wa     (wa folded in as per-partition scale)
  q_nat[j,hd] = q @ Wq + bq                (bias via K=1 ones x bq matmul)
  S_T[j, i]   = qs_T.T @ v_T  per head     (score transposed; softmax dim = free)
  E_T         = exp(S_T)                   (no max-subtraction: |S|<~0.2, and the
                                            +ba shift cancels in softmax anyway)
  r[j]        = 1 / sum_i E_T[j, i]        (free-dim reduce + reciprocal)
  att[i, j]   = E_T.T * r                  (scale then PE transpose)
  ctx_T[d, i] = (q_nat * r).T @ E_T        (deferred normalization folded into q)
  head[hd]    = sum_i v_T * ctx_T          (fused multiply + free-dim accum)
  fused       = head @ Wo + bo             (fp16 matmuls, K=1 bias matmul)

Matmul operands are fp16 (validated: att absmax rel err ~9e-4, fused ~6e-4);
accumulation, softmax and outputs are fp32.
"""

import numpy as np

H_OUT = 2
H_DIM = 256
NV = 128
NQ = 512
KD = 128          # V_DIM == Q_DIM
HD = H_OUT * H_DIM
N_CORES = 8

_CACHE = {}


def _build(stage=99):
    import concourse.bacc as bacc
    import concourse.tile as tile
    from concourse import mybir
    from concourse.masks import make_identity

    F32 = mybir.dt.float32
    F16 = mybir.dt.float16
    Identity = mybir.ActivationFunctionType.Identity
    Exp = mybir.ActivationFunctionType.Exp
    MULT = mybir.AluOpType.mult
    ADD = mybir.AluOpType.add
    BYPASS = mybir.AluOpType.bypass
    AX = mybir.AxisListType.X

    nc = bacc.Bacc("TRN2", target_bir_lowering=False, debug=False)

    v_d = nc.dram_tensor("v", [NV, KD], F32, kind="ExternalInput")
    q_d = nc.dram_tensor("q", [NQ, KD], F32, kind="ExternalInput")
    Wv_d = nc.dram_tensor("Wv", [KD, HD], F32, kind="ExternalInput")
    Wq_d = nc.dram_tensor("Wq", [KD, HD], F32, kind="ExternalInput")
    Wo_d = nc.dram_tensor("Wo", [HD, H_DIM], F32, kind="ExternalInput")
    bv_d = nc.dram_tensor("bv", [1, HD], F32, kind="ExternalInput")
    bq_d = nc.dram_tensor("bq", [1, HD], F32, kind="ExternalInput")
    wa_d = nc.dram_tensor("wa", [1, H_DIM], F32, kind="ExternalInput")
    bo_d = nc.dram_tensor("bo", [1, H_DIM], F32, kind="ExternalInput")

    fused_d = nc.dram_tensor("fused", [1, H_DIM], F32, kind="ExternalOutput")
    att_d = nc.dram_tensor("att", [H_OUT, NV, NQ], F32, kind="ExternalOutput")

    def emit(tc, consts, work, ptiny, pbig, pquad, pfused):
        # ---------------- loads ----------------
        Wq_sb = work.tile([KD, HD], F32)
        nc.sync.dma_start(out=Wq_sb[:], in_=Wq_d[:])
        Wv_sb = work.tile([KD, HD], F32)
        nc.sync.dma_start(out=Wv_sb[:], in_=Wv_d[:])
        q_sb = work.tile([128, NQ], F32)  # q_sb[p, c*128+k] = q[c*128+p, k]
        nc.sync.dma_start(
            out=q_sb[:].rearrange("p (c k) -> p c k", k=128),
            in_=q_d[:].rearrange("(c p) k -> p c k", p=128),
        )
        v_sb = work.tile([NV, KD], F32)
        nc.sync.dma_start(out=v_sb[:], in_=v_d[:])
        Wo_sb = work.tile([128, 4 * H_DIM], F32)  # col block c = Wo[c*128:(c+1)*128, :]
        nc.sync.dma_start(
            out=Wo_sb[:].rearrange("p (c o) -> p c o", o=H_DIM),
            in_=Wo_d[:].rearrange("(c p) o -> p c o", p=128),
        )
        bv_r = work.tile([1, HD], F32)
        nc.sync.dma_start(out=bv_r[:], in_=bv_d[:])
        bq_r = work.tile([1, HD], F32)
        nc.sync.dma_start(out=bq_r[:], in_=bq_d[:])
        wa_r = work.tile([1, H_DIM], F32)
        nc.sync.dma_start(out=wa_r[:], in_=wa_d[:])
        bo_r = work.tile([1, H_DIM], F32)
        nc.sync.dma_start(out=bo_r[:], in_=bo_d[:])

        # ---------------- constants ----------------
        I32 = consts.tile([128, 128], F32)
        make_identity(nc, I32[:])
        I16 = consts.tile([128, 128], F16)
        make_identity(nc, I16[:])
        ones16 = consts.tile([1, 128], F16)
        nc.vector.memset(ones16[:], 1.0)
        one11 = consts.tile([1, 1], F32)
        nc.vector.memset(one11[:], 1.0)

        # per-partition constants via K=1 outer-product "transposes":
        # psum[:, col] = row_slice.T
        pt = ptiny.tile([128, 10], F32)
        for c in range(4):
            nc.tensor.matmul(pt[:, c : c + 1], bv_r[0:1, c * 128 : (c + 1) * 128], one11[:])
        for c in range(4):
            nc.tensor.matmul(pt[:, 4 + c : 5 + c], bq_r[0:1, c * 128 : (c + 1) * 128], one11[:])
        for c in range(2):
            nc.tensor.matmul(pt[:, 8 + c : 9 + c], wa_r[0:1, c * 128 : (c + 1) * 128], one11[:])
        bvq_T = work.tile([128, 8], F32)  # cols 0-3 bv_T, 4-7 bq_T
        nc.vector.tensor_copy(bvq_T[:], pt[:, 0:8])
        wa_T4 = work.tile([128, 4], F32)  # wa_T for chunk c is col c (duplicated per head)
        nc.vector.tensor_copy(wa_T4[:, 0:2], pt[:, 8:10])
        nc.vector.tensor_copy(wa_T4[:, 2:4], pt[:, 8:10])
        bias_q = work.tile([128, 4], F32)  # (bq * wa) transposed, per chunk
        nc.vector.tensor_tensor(out=bias_q[:], in0=bvq_T[:, 4:8], in1=wa_T4[:], op=MULT)

        # fp16 casts of weights / bias rows
        Wq16 = work.tile([KD, HD], F16)
        nc.gpsimd.tensor_copy(Wq16[:], Wq_sb[:])
        Wv16 = work.tile([KD, HD], F16)
        nc.gpsimd.tensor_copy(Wv16[:], Wv_sb[:])
        bq16_r = work.tile([1, HD], F16)
        nc.vector.tensor_copy(bq16_r[:], bq_r[:])
        bo16_r = work.tile([1, H_DIM], F16)
        nc.vector.tensor_copy(bo16_r[:], bo_r[:])
        Wo16 = work.tile([128, 4 * H_DIM], F16)
        nc.gpsimd.tensor_copy(Wo16[:], Wo_sb[:])

        def _dump(src_ap, n=128):
            """Debug: write a [128, n] view of src into att_d[0]."""
            dbg = work.tile([128, NQ], F32, tag="att32")
            nc.scalar.copy(dbg[:, 0:n], src_ap)
            nc.sync.dma_start(out=att_d[0], in_=dbg[:])

        # ---------------- input transposes ----------------
        qbT16 = work.tile([KD, NQ], F16)  # [k, j]
        for jc in range(4):
            tp = pquad.tile([128, 4, 128], F32, tag="quad")
            nc.tensor.transpose(tp[:, 0, :], q_sb[:, jc * 128 : (jc + 1) * 128], I32[:])
            nc.scalar.copy(qbT16[:, jc * 128 : (jc + 1) * 128], tp[:, 0, :])
        vbT16 = work.tile([KD, NV], F16)  # [k, i]
        tp = pquad.tile([128, 4, 128], F32, tag="quad")
        nc.tensor.transpose(tp[:, 0, :], v_sb[:], I32[:])
        nc.scalar.copy(vbT16[:], tp[:, 0, :])

        if stage <= 1:
            _dump(qbT16[:, 0:128])
            return

        # ---------------- projections ----------------
        # qs_T[d, c, j]: wa-scaled, bias-included, fp16
        qs_T = work.tile([128, 4 * NQ], F16)
        for c in range(4):
            qp = pbig.tile([128, NQ], F32, tag="big")
            nc.tensor.matmul(qp[:], Wq16[:, c * 128 : (c + 1) * 128], qbT16[:])
            nc.vector.tensor_scalar(
                out=qs_T[:, c * NQ : (c + 1) * NQ],
                in0=qp[:],
                scalar1=wa_T4[:, c : c + 1],
                scalar2=bias_q[:, c : c + 1],
                op0=MULT,
                op1=ADD,
            )

        # v_T16[d, c, i]: bias-included, fp16
        v_T16 = work.tile([128, 4 * NV], F16)
        vp = pquad.tile([128, 4, 128], F32, tag="quad")
        for c in range(4):
            nc.tensor.matmul(vp[:, c, :], Wv16[:, c * 128 : (c + 1) * 128], vbT16[:])
        for c in range(4):
            nc.scalar.activation(
                v_T16[:, c * NV : (c + 1) * NV],
                vp[:, c, :],
                Identity,
                bias=bvq_T[:, c : c + 1],
                scale=1.0,
            )

        if stage <= 2:
            _dump(v_T16[:, 0:128])
            return

        # ---------------- score + softmax (transposed layout) ----------------
        E_T16 = work.tile([128, 8 * 128], F16)  # [j, (h,jc), i]
        colsT = work.tile([128, 8], F32)
        for h in range(2):
            sp = pquad.tile([128, 4, 128], F32, tag="quad")
            for jc in range(4):
                for dc in range(2):
                    c = h * 2 + dc
                    nc.tensor.matmul(
                        sp[:, jc, :],
                        qs_T[:, c * NQ + jc * 128 : c * NQ + (jc + 1) * 128],
                        v_T16[:, c * NV : (c + 1) * NV],
                        start=(dc == 0),
                        stop=(dc == 1),
                    )
            for jc in range(4):
                hc = h * 4 + jc
                nc.scalar.activation(
                    E_T16[:, hc * 128 : (hc + 1) * 128], sp[:, jc, :], Exp
                )
                nc.vector.reduce_sum(
                    out=colsT[:, hc : hc + 1],
                    in_=E_T16[:, hc * 128 : (hc + 1) * 128],
                    axis=AX,
                )
        r_T = work.tile([128, 8], F32)
        nc.vector.reciprocal(r_T[:], colsT[:])

        if stage <= 3:
            _dump(E_T16[:, 0:128])
            return

        # ---------------- q_nat + deferred-normalization scale ----------------
        q_sc = work.tile([128, 4 * HD], F16)  # [j, jc, hd]
        for jc in range(4):
            qnp = pbig.tile([128, HD], F32, tag="big")
            nc.tensor.matmul(
                qnp[:], qbT16[:, jc * 128 : (jc + 1) * 128], Wq16[:], start=True, stop=False
            )
            nc.tensor.matmul(qnp[:], ones16[:], bq16_r[:], start=False, stop=True)
            for h in range(2):
                nc.vector.tensor_scalar(
                    out=q_sc[:, jc * HD + h * H_DIM : jc * HD + (h + 1) * H_DIM],
                    in0=qnp[:, h * H_DIM : (h + 1) * H_DIM],
                    scalar1=r_T[:, h * 4 + jc : h * 4 + jc + 1],
                    scalar2=None,
                    op0=MULT,
                )

        if stage <= 4:
            _dump(q_sc[:, 0:128])
            return

        # ---------------- ctx (transposed) + head ----------------
        ctxp = pquad.tile([128, 4, 128], F32, tag="quad")
        for h in range(2):
            for dh in range(2):
                c = h * 2 + dh
                for jc in range(4):
                    nc.tensor.matmul(
                        ctxp[:, c, :],
                        q_sc[:, jc * HD + c * 128 : jc * HD + (c + 1) * 128],
                        E_T16[:, (h * 4 + jc) * 128 : (h * 4 + jc + 1) * 128],
                        start=(jc == 0),
                        stop=(jc == 3),
                    )
        headT = work.tile([128, 4], F32)
        prod = work.tile([128, 128], F32)
        for c in range(4):
            nc.vector.scalar_tensor_tensor(
                out=prod[:],
                in0=ctxp[:, c, :],
                scalar=1.0,
                in1=v_T16[:, c * NV : (c + 1) * NV],
                op0=BYPASS,
                op1=MULT,
                accum_out=headT[:, c : c + 1],
            )
        headT16 = work.tile([128, 4], F16)
        nc.vector.tensor_copy(headT16[:], headT[:])

        if stage <= 5:
            _dump(headT[:, 0:4], n=4)
            return

        # ---------------- fused output ----------------
        fp = pfused.tile([1, H_DIM], F32)
        for c in range(4):
            nc.tensor.matmul(
                fp[:],
                headT16[:, c : c + 1],
                Wo16[:, c * H_DIM : (c + 1) * H_DIM],
                start=(c == 0),
                stop=False,
            )
        nc.tensor.matmul(
            fp[:], ones16[0:1, 0:1], bo16_r[:], start=False, stop=True
        )
        fused_sb = work.tile([1, H_DIM], F32)
        nc.scalar.copy(fused_sb[:], fp[:])
        nc.sync.dma_start(out=fused_d[:], in_=fused_sb[:])

        if stage <= 6:
            return

        # ---------------- att output ----------------
        attT32 = work.tile([128, 8 * 128], F32)
        for hc in range(8):
            nc.gpsimd.tensor_scalar(
                out=attT32[:, hc * 128 : (hc + 1) * 128],
                in0=E_T16[:, hc * 128 : (hc + 1) * 128],
                scalar1=r_T[:, hc : hc + 1],
                scalar2=None,
                op0=MULT,
            )
        if stage <= 7:
            _dump(attT32[:, 0:128])
            return

        for h in range(2):
            atp = pquad.tile([128, 4, 128], F32, tag="quad")
            for jc in range(4):
                nc.tensor.transpose(
                    atp[:, jc, :],
                    attT32[:, (h * 4 + jc) * 128 : (h * 4 + jc + 1) * 128],
                    I32[:],
                )
            att32 = work.tile([128, NQ], F32, tag="att32")
            for jc in range(4):
                nc.scalar.copy(att32[:, jc * 128 : (jc + 1) * 128], atp[:, jc, :])
            nc.sync.dma_start(out=att_d[h], in_=att32[:])

    with tile.TileContext(nc) as tc:
        with (
            tc.tile_pool(name="consts", bufs=1) as consts,
            tc.tile_pool(name="work", bufs=1) as work,
            tc.tile_pool(name="ptiny", bufs=1, space="PSUM") as ptiny,
            tc.tile_pool(name="pbig", bufs=2, space="PSUM") as pbig,
            tc.tile_pool(name="pquad", bufs=2, space="PSUM") as pquad,
            tc.tile_pool(name="pfused", bufs=1, space="PSUM") as pfused,
        ):
            emit(tc, consts, work, ptiny, pbig, pquad, pfused)

    nc.compile()
    return nc


def get_nc(stage=99):
    key = ("nc", stage)
    if key not in _CACHE:
        _CACHE[key] = _build(stage)
    return _CACHE[key]


def kernel(v, q, Wv, bv, Wq, bq, wa, ba, Wo, bo):
    from concourse.bass_utils import run_bass_kernel_spmd

    nc = get_nc()
    v = np.ascontiguousarray(np.asarray(v, dtype=np.float32))
    q = np.ascontiguousarray(np.asarray(q, dtype=np.float32))
    common = {
        "Wv": np.ascontiguousarray(np.asarray(Wv, np.float32)),
        "Wq": np.ascontiguousarray(np.asarray(Wq, np.float32)),
        "Wo": np.ascontiguousarray(np.asarray(Wo, np.float32)),
        "bv": np.asarray(bv, np.float32).reshape(1, HD),
        "bq": np.asarray(bq, np.float32).reshape(1, HD),
        "wa": np.asarray(wa, np.float32).reshape(1, H_DIM),
        "bo": np.asarray(bo, np.float32).reshape(1, H_DIM),
    }
    B = v.shape[0]
    assert B == N_CORES, f"expected batch {N_CORES}, got {B}"
    in_maps = [dict(common, v=v[b], q=q[b]) for b in range(B)]
    res = run_bass_kernel_spmd(nc, in_maps, core_ids=list(range(N_CORES)))
    fused = np.concatenate([r["fused"] for r in res.results], axis=0)
    att = np.stack([r["att"] for r in res.results], axis=0)
    att = att.reshape(B, H_OUT, NV * NQ, 1)
    return fused, att


# revision 18
# speedup vs baseline: 1.4644x; 1.4644x over previous
"""BANLayer Trainium2 kernel.

Data-parallel over batch: 8 examples -> 8 NeuronCores, weights replicated.

The host wrapper does layout-only prep (transposes / fp16 casts / bias-vector
reshapes — no FLOPs on activations beyond dtype rounding); the device does all
the math:

Per-core (one example; i=num_v=128, j=num_q=512, k=in_dim=128,
hd=H_OUT*H_DIM=512; chunk c in 0..3 <-> (h, dhalf)):

  q_T[hd, j]  = Wq.T @ q.T                  (4 MMs, N=512)
  qs_T        = q_T * wa + (bq*wa)          (DVE scale+bias, per-partition, fp16)
  v_T[hd, i]  = Wv.T @ v.T (+bv via ACT)    (4 MMs, N=128)
  S_T[j, i]   = qs_T.T @ v_T  per head      (16 MMs; softmax dim = free dim;
                                             no max-subtraction: |S|<~0.2 and
                                             the +ba shift cancels in softmax)
  E_T, colsum = ACT Exp with accum_out      (8 ACT ops, colsum for free)
  r[j]        = 1/colsum                    (DVE reciprocal, [128,8])
  E_sc        = E_T * r                     (= att transposed; fp16; DMA'd out
                                             and un-transposed on host)
  q_nat[j,hd] = q @ Wq + bq                 (4+4 MMs; bias via K=1 ones x bq)
  ctx_T[d, i] = q_nat.T @ E_sc              (16 MMs; deferred normalization in E_sc)
  head[hd]    = sum_i v_T * ctx_T           (DVE fused mult + free-dim accum)
  fused       = head @ Wo + bo              (5 fp16 MMs)

fp16 matmul operands, fp32 accumulation/softmax/outputs.
"""

import numpy as np

H_OUT = 2
H_DIM = 256
NV = 128
NQ = 512
KD = 128          # V_DIM == Q_DIM
HD = H_OUT * H_DIM
N_CORES = 8

_CACHE = {}


def _build(stage=99):
    import concourse.bacc as bacc
    import concourse.tile as tile
    from concourse import mybir

    F32 = mybir.dt.float32
    F16 = mybir.dt.float16
    Identity = mybir.ActivationFunctionType.Identity
    Exp = mybir.ActivationFunctionType.Exp
    MULT = mybir.AluOpType.mult
    ADD = mybir.AluOpType.add
    BYPASS = mybir.AluOpType.bypass

    nc = bacc.Bacc("TRN2", target_bir_lowering=False, debug=False)

    # layout-prepped inputs (host does transposes/casts, see _prep_* below)
    qT_d = nc.dram_tensor("qT16", [KD, NQ], F16, kind="ExternalInput")
    vT_d = nc.dram_tensor("vT16", [KD, NV], F16, kind="ExternalInput")
    Wq_d = nc.dram_tensor("Wq16", [KD, HD], F16, kind="ExternalInput")
    Wv_d = nc.dram_tensor("Wv16", [KD, HD], F16, kind="ExternalInput")
    Wo_d = nc.dram_tensor("Wo16", [128, 4 * H_DIM], F16, kind="ExternalInput")
    ct_d = nc.dram_tensor("CT32", [128, 12], F32, kind="ExternalInput")
    bq_d = nc.dram_tensor("bq16", [1, HD], F16, kind="ExternalInput")
    bo_d = nc.dram_tensor("bo16", [1, H_DIM], F16, kind="ExternalInput")

    fused_d = nc.dram_tensor("fused", [1, H_DIM], F32, kind="ExternalOutput")
    attT_d = nc.dram_tensor("attT16", [128, 8 * 128], F16, kind="ExternalOutput")

    def emit(consts, work, pbig, pquad, pfused):
        # ---------------- loads ----------------
        Wq16 = work.tile([KD, HD], F16)
        nc.sync.dma_start(out=Wq16[:], in_=Wq_d[:])
        qT16 = work.tile([KD, NQ], F16)
        nc.sync.dma_start(out=qT16[:], in_=qT_d[:])
        Wv16 = work.tile([KD, HD], F16)
        nc.sync.dma_start(out=Wv16[:], in_=Wv_d[:])
        vT16 = work.tile([KD, NV], F16)
        nc.sync.dma_start(out=vT16[:], in_=vT_d[:])
        Wo16 = work.tile([128, 4 * H_DIM], F16)
        nc.sync.dma_start(out=Wo16[:], in_=Wo_d[:])
        CT = work.tile([128, 12], F32)  # 0-3 bv_T, 4-7 (bq*wa)_T, 8-11 wa_T4
        nc.sync.dma_start(out=CT[:], in_=ct_d[:])
        bq16_r = work.tile([1, HD], F16)
        nc.sync.dma_start(out=bq16_r[:], in_=bq_d[:])
        bo16_r = work.tile([1, H_DIM], F16)
        nc.sync.dma_start(out=bo16_r[:], in_=bo_d[:])

        ones16 = consts.tile([1, 128], F16)
        nc.vector.memset(ones16[:], 1.0)

        # ---------------- q_T -> qs_T ----------------
        qs_T = work.tile([128, 4 * NQ], F16)  # [d, c, j]
        for c in range(4):
            qp = pbig.tile([128, NQ], F32, tag="big")
            nc.tensor.matmul(qp[:], Wq16[:, c * 128 : (c + 1) * 128], qT16[:])
            nc.vector.tensor_scalar(
                out=qs_T[:, c * NQ : (c + 1) * NQ],
                in0=qp[:],
                scalar1=CT[:, 8 + c : 9 + c],
                scalar2=CT[:, 4 + c : 5 + c],
                op0=MULT,
                op1=ADD,
            )

        # ---------------- q_nat (independent of softmax path) ----------------
        q_nat16 = work.tile([128, 4 * HD], F16)  # [j, jc, hd]
        for jc in range(4):
            qnp = pbig.tile([128, HD], F32, tag="big")
            nc.tensor.matmul(
                qnp[:], qT16[:, jc * 128 : (jc + 1) * 128], Wq16[:], start=True, stop=False
            )
            nc.tensor.matmul(qnp[:], ones16[:], bq16_r[:], start=False, stop=True)
            nc.scalar.copy(q_nat16[:, jc * HD : (jc + 1) * HD], qnp[:])

        # ---------------- v_T ----------------
        v_T16 = work.tile([128, 4 * NV], F16)  # [d, c, i]
        vp = pquad.tile([128, 4, 128], F32, tag="quad")
        for c in range(4):
            nc.tensor.matmul(vp[:, c, :], Wv16[:, c * 128 : (c + 1) * 128], vT16[:])
        for c in range(4):
            nc.scalar.activation(
                v_T16[:, c * NV : (c + 1) * NV],
                vp[:, c, :],
                Identity,
                bias=CT[:, c : c + 1],
                scale=1.0,
            )

        if stage <= 1:
            return

        # ---------------- score + softmax (transposed layout) ----------------
        E_T16 = work.tile([128, 8 * 128], F16)  # [j, (h,jc), i]
        colsT = work.tile([128, 8], F32)
        for h in range(2):
            sp = pquad.tile([128, 4, 128], F32, tag="quad")
            for jc in range(4):
                for dc in range(2):
                    c = h * 2 + dc
                    nc.tensor.matmul(
                        sp[:, jc, :],
                        qs_T[:, c * NQ + jc * 128 : c * NQ + (jc + 1) * 128],
                        v_T16[:, c * NV : (c + 1) * NV],
                        start=(dc == 0),
                        stop=(dc == 1),
                    )
            for jc in range(4):
                hc = h * 4 + jc
                nc.scalar.activation(
                    E_T16[:, hc * 128 : (hc + 1) * 128],
                    sp[:, jc, :],
                    Exp,
                    accum_out=colsT[:, hc : hc + 1],
                )
        r_T = work.tile([128, 8], F32)
        nc.vector.reciprocal(r_T[:], colsT[:])

        # E_sc = att transposed (fp16) — DMA'd out; also the ctx rhs
        E_sc16 = work.tile([128, 8 * 128], F16)
        for hc in range(8):
            nc.vector.tensor_scalar(
                out=E_sc16[:, hc * 128 : (hc + 1) * 128],
                in0=E_T16[:, hc * 128 : (hc + 1) * 128],
                scalar1=r_T[:, hc : hc + 1],
                scalar2=None,
                op0=MULT,
            )
        nc.sync.dma_start(out=attT_d[:], in_=E_sc16[:])

        if stage <= 2:
            return

        # ---------------- ctx (transposed) + head ----------------
        ctxp = pquad.tile([128, 4, 128], F32, tag="quad")
        for h in range(2):
            for dh in range(2):
                c = h * 2 + dh
                for jc in range(4):
                    nc.tensor.matmul(
                        ctxp[:, c, :],
                        q_nat16[:, jc * HD + c * 128 : jc * HD + (c + 1) * 128],
                        E_sc16[:, (h * 4 + jc) * 128 : (h * 4 + jc + 1) * 128],
                        start=(jc == 0),
                        stop=(jc == 3),
                    )
        headT = work.tile([128, 4], F32)
        prod = work.tile([128, 128], F32)
        for c in range(4):
            nc.vector.scalar_tensor_tensor(
                out=prod[:],
                in0=ctxp[:, c, :],
                scalar=1.0,
                in1=v_T16[:, c * NV : (c + 1) * NV],
                op0=BYPASS,
                op1=MULT,
                accum_out=headT[:, c : c + 1],
            )
        headT16 = work.tile([128, 4], F16)
        nc.vector.tensor_copy(headT16[:], headT[:])

        # ---------------- fused output ----------------
        fp = pfused.tile([1, H_DIM], F32)
        for c in range(4):
            nc.tensor.matmul(
                fp[:],
                headT16[:, c : c + 1],
                Wo16[:, c * H_DIM : (c + 1) * H_DIM],
                start=(c == 0),
                stop=False,
            )
        nc.tensor.matmul(fp[:], ones16[0:1, 0:1], bo16_r[:], start=False, stop=True)
        fused_sb = work.tile([1, H_DIM], F32)
        nc.scalar.copy(fused_sb[:], fp[:])
        nc.sync.dma_start(out=fused_d[:], in_=fused_sb[:])

    with tile.TileContext(nc) as tc:
        with (
            tc.tile_pool(name="consts", bufs=1) as consts,
            tc.tile_pool(name="work", bufs=1) as work,
            tc.tile_pool(name="pbig", bufs=2, space="PSUM") as pbig,
            tc.tile_pool(name="pquad", bufs=2, space="PSUM") as pquad,
            tc.tile_pool(name="pfused", bufs=1, space="PSUM") as pfused,
        ):
            emit(consts, work, pbig, pquad, pfused)

    nc.compile()
    return nc


def get_nc(stage=99):
    key = ("nc", stage)
    if key not in _CACHE:
        _CACHE[key] = _build(stage)
    return _CACHE[key]


def prep_weights(Wv, bv, Wq, bq, wa, Wo, bo):
    """Host-side layout prep of the replicated weights (pure layout/dtype)."""
    f16, f32 = np.float16, np.float32
    Wq16 = np.ascontiguousarray(np.asarray(Wq, f32)).astype(f16)
    Wv16 = np.ascontiguousarray(np.asarray(Wv, f32)).astype(f16)
    Wo = np.asarray(Wo, f32)
    Wo16 = np.ascontiguousarray(
        np.transpose(Wo.reshape(4, 128, H_DIM), (1, 0, 2)).reshape(128, 4 * H_DIM)
    ).astype(f16)
    bv_T = np.asarray(bv, f32).reshape(4, 128).T            # [128, 4]
    bq_T = np.asarray(bq, f32).reshape(4, 128).T            # [128, 4]
    wa_T = np.asarray(wa, f32).reshape(2, 128).T            # [128, 2]
    wa_T4 = np.concatenate([wa_T, wa_T], axis=1)            # [128, 4]
    CT32 = np.ascontiguousarray(
        np.concatenate([bv_T, bq_T * wa_T4, wa_T4], axis=1).astype(f32)
    )
    bq16 = np.asarray(bq, f32).reshape(1, HD).astype(f16)
    bo16 = np.asarray(bo, f32).reshape(1, H_DIM).astype(f16)
    return {
        "Wq16": Wq16,
        "Wv16": Wv16,
        "Wo16": Wo16,
        "CT32": CT32,
        "bq16": bq16,
        "bo16": bo16,
    }


def prep_example(v_b, q_b):
    """Host-side layout prep of one example (transpose + fp16 cast)."""
    f16 = np.float16
    return {
        "vT16": np.ascontiguousarray(np.asarray(v_b, np.float32).T).astype(f16),
        "qT16": np.ascontiguousarray(np.asarray(q_b, np.float32).T).astype(f16),
    }


def unprep_att(attT16):
    """[128 j, 8 (h,jc), 128 i] fp16 -> [2, 128 i, 512 j] fp32."""
    a = np.asarray(attT16).reshape(128, 2, 4, 128).astype(np.float32)
    return np.transpose(a, (1, 3, 2, 0)).reshape(H_OUT, NV, NQ)


def kernel(v, q, Wv, bv, Wq, bq, wa, ba, Wo, bo):
    from concourse.bass_utils import run_bass_kernel_spmd

    nc = get_nc()
    common = prep_weights(Wv, bv, Wq, bq, wa, Wo, bo)
    B = np.asarray(v).shape[0]
    assert B == N_CORES, f"expected batch {N_CORES}, got {B}"
    in_maps = [dict(common, **prep_example(v[b], q[b])) for b in range(B)]
    res = run_bass_kernel_spmd(nc, in_maps, core_ids=list(range(N_CORES)))
    fused = np.concatenate([r["fused"] for r in res.results], axis=0)
    att = np.stack([unprep_att(r["attT16"]) for r in res.results], axis=0)
    att = att.reshape(B, H_OUT, NV * NQ, 1)
    return fused, att


# revision 20
# speedup vs baseline: 1.7470x; 1.1930x over previous
"""BANLayer Trainium2 kernel.

Data-parallel over batch: 8 examples -> 8 NeuronCores, weights replicated.

The host wrapper does layout-only prep (transposes / fp16 casts / bias-vector
reshapes — no FLOPs on activations beyond dtype rounding); the device does all
the math:

Per-core (one example; i=num_v=128, j=num_q=512, k=in_dim=128,
hd=H_OUT*H_DIM=512; chunk c in 0..3 <-> (h, dhalf)):

  q_T[hd, j]  = Wq.T @ q.T                  (4 MMs, N=512)
  qs_T        = q_T * wa + (bq*wa)          (DVE scale+bias, per-partition, fp16)
  v_T[hd, i]  = Wv.T @ v.T (+bv via ACT)    (4 MMs, N=128)
  S_T[j, i]   = qs_T.T @ v_T  per head      (16 MMs; softmax dim = free dim;
                                             no max-subtraction: |S|<~0.2 and
                                             the +ba shift cancels in softmax)
  E_T, colsum = ACT Exp with accum_out      (8 ACT ops, colsum for free)
  r[j]        = 1/colsum                    (DVE reciprocal, per head)
  E_sc        = E_T * r                     (= att transposed; fp16; DMA'd out
                                             and un-transposed on host)
  q_nat[j,hd] = q @ Wq + bq                 (4+4 MMs; bias via K=1 ones x bq)
  ctx_T[d, i] = q_nat.T @ E_sc              (16 MMs; deferred normalization in E_sc)
  head[hd]    = sum_i v_T * ctx_T           (DVE fused mult + free-dim accum)
  fused       = head @ Wo + bo              (5 fp16 MMs)

fp16 matmul operands, fp32 accumulation/softmax/outputs.

Big fp16 operands are shipped as one packed DRAM tensor to minimize DMA count
(fewer semaphores -> shorter kernel prologue/epilogue).
"""

import numpy as np

H_OUT = 2
H_DIM = 256
NV = 128
NQ = 512
KD = 128          # V_DIM == Q_DIM
HD = H_OUT * H_DIM
N_CORES = 8

# packed fp16 input layout: [128, PK_COLS]
PK_WQ = 0                     # Wq16           [128, 512]
PK_QT = PK_WQ + HD            # qT16           [128, 512]
PK_WV = PK_QT + NQ            # Wv16           [128, 512]
PK_VT = PK_WV + HD            # vT16           [128, 128]
PK_WO = PK_VT + NV            # Wo16 (chunked) [128, 1024]
PK_COLS = PK_WO + 4 * H_DIM   # 2688

_CACHE = {}


def _build(stage=99):
    import concourse.bacc as bacc
    import concourse.tile as tile
    from concourse import mybir

    F32 = mybir.dt.float32
    F16 = mybir.dt.float16
    Identity = mybir.ActivationFunctionType.Identity
    Exp = mybir.ActivationFunctionType.Exp
    MULT = mybir.AluOpType.mult
    ADD = mybir.AluOpType.add
    BYPASS = mybir.AluOpType.bypass

    nc = bacc.Bacc("TRN2", target_bir_lowering=False, debug=False)

    ct_d = nc.dram_tensor("CT32", [128, 12], F32, kind="ExternalInput")
    pk_d = nc.dram_tensor("PK16", [128, PK_COLS], F16, kind="ExternalInput")
    rows_d = nc.dram_tensor("ROWS16", [1, HD + H_DIM], F16, kind="ExternalInput")

    fused_d = nc.dram_tensor("fused", [1, H_DIM], F32, kind="ExternalOutput")
    attT_d = nc.dram_tensor("attT16", [128, 8 * 128], F16, kind="ExternalOutput")

    def emit(consts, work, pbig, pquad, pfused):
        # ---------------- loads (CT first: needed by first DVE op) ----------------
        CT = work.tile([128, 12], F32)  # 0-3 bv_T, 4-7 (bq*wa)_T, 8-11 wa_T4
        nc.sync.dma_start(out=CT[:], in_=ct_d[:])
        # split the pack into two DMAs so the q-side lands first
        pk = work.tile([128, PK_COLS], F16)
        nc.sync.dma_start(out=pk[:, : PK_WV], in_=pk_d[:, : PK_WV])
        nc.sync.dma_start(out=pk[:, PK_WV :], in_=pk_d[:, PK_WV :])
        rows = work.tile([1, HD + H_DIM], F16)
        nc.sync.dma_start(out=rows[:], in_=rows_d[:])

        Wq16 = pk[:, PK_WQ : PK_WQ + HD]
        qT16 = pk[:, PK_QT : PK_QT + NQ]
        Wv16 = pk[:, PK_WV : PK_WV + HD]
        vT16 = pk[:, PK_VT : PK_VT + NV]
        Wo16 = pk[:, PK_WO : PK_WO + 4 * H_DIM]
        bq16_r = rows[:, :HD]
        bo16_r = rows[:, HD:]

        ones16 = consts.tile([1, 128], F16)
        nc.vector.memset(ones16[:], 1.0)
        zeros1 = consts.tile([128, 1], F32)
        nc.vector.memset(zeros1[:], 0.0)

        # ---------------- q_T -> qs_T ----------------
        qs_T = work.tile([128, 4 * NQ], F16)  # [d, c, j]
        for c in range(4):
            qp = pbig.tile([128, NQ], F32, tag="big")
            nc.tensor.matmul(qp[:], Wq16[:, c * 128 : (c + 1) * 128], qT16)
            nc.vector.tensor_scalar(
                out=qs_T[:, c * NQ : (c + 1) * NQ],
                in0=qp[:],
                scalar1=CT[:, 8 + c : 9 + c],
                scalar2=CT[:, 4 + c : 5 + c],
                op0=MULT,
                op1=ADD,
            )

        # ---------------- q_nat (independent of softmax path) ----------------
        q_nat16 = work.tile([128, 4 * HD], F16)  # [j, jc, hd]
        for jc in range(4):
            qnp = pbig.tile([128, HD], F32, tag="big")
            nc.tensor.matmul(
                qnp[:], qT16[:, jc * 128 : (jc + 1) * 128], Wq16, start=True, stop=False
            )
            nc.tensor.matmul(qnp[:], ones16[:], bq16_r, start=False, stop=True)
            nc.vector.tensor_copy(q_nat16[:, jc * HD : (jc + 1) * HD], qnp[:])

        # ---------------- v_T ----------------
        v_T16 = work.tile([128, 4 * NV], F16)  # [d, c, i]
        vp = pquad.tile([128, 4, 128], F32, tag="quad")
        for c in range(4):
            nc.tensor.matmul(vp[:, c, :], Wv16[:, c * 128 : (c + 1) * 128], vT16)
        for c in range(4):
            nc.scalar.activation(
                v_T16[:, c * NV : (c + 1) * NV],
                vp[:, c, :],
                Identity,
                bias=CT[:, c : c + 1],
                scale=1.0,
            )

        if stage <= 1:
            return

        # ---------------- score + softmax (transposed layout) ----------------
        E_T16 = work.tile([128, 8 * 128], F16)  # [j, (h,jc), i]
        E_sc16 = work.tile([128, 8 * 128], F16)
        colsT = work.tile([128, 8], F32)
        r_T = work.tile([128, 8], F32)
        for h in range(2):
            sp = pquad.tile([128, 4, 128], F32, tag="quad")
            for jc in range(4):
                for dc in range(2):
                    c = h * 2 + dc
                    nc.tensor.matmul(
                        sp[:, jc, :],
                        qs_T[:, c * NQ + jc * 128 : c * NQ + (jc + 1) * 128],
                        v_T16[:, c * NV : (c + 1) * NV],
                        start=(dc == 0),
                        stop=(dc == 1),
                    )
            for jc in range(4):
                hc = h * 4 + jc
                nc.scalar.activation(
                    E_T16[:, hc * 128 : (hc + 1) * 128],
                    sp[:, jc, :],
                    Exp,
                    bias=zeros1[:],
                    accum_out=colsT[:, hc : hc + 1],
                )
            # per-head reciprocal so head h's ctx can start while the other
            # head is still in its score/exp phase
            nc.vector.reciprocal(r_T[:, h * 4 : (h + 1) * 4], colsT[:, h * 4 : (h + 1) * 4])
            for jc in range(4):
                hc = h * 4 + jc
                nc.vector.tensor_scalar(
                    out=E_sc16[:, hc * 128 : (hc + 1) * 128],
                    in0=E_T16[:, hc * 128 : (hc + 1) * 128],
                    scalar1=r_T[:, hc : hc + 1],
                    scalar2=None,
                    op0=MULT,
                )
        nc.sync.dma_start(out=attT_d[:], in_=E_sc16[:])

        if stage <= 2:
            return

        # ---------------- ctx (transposed) + head + fused ----------------
        ctxp = pquad.tile([128, 4, 128], F32, tag="quad")
        for h in range(2):
            for dh in range(2):
                c = h * 2 + dh
                for jc in range(4):
                    nc.tensor.matmul(
                        ctxp[:, c, :],
                        q_nat16[:, jc * HD + c * 128 : jc * HD + (c + 1) * 128],
                        E_sc16[:, (h * 4 + jc) * 128 : (h * 4 + jc + 1) * 128],
                        start=(jc == 0),
                        stop=(jc == 3),
                    )
        headT = work.tile([128, 4], F32)
        headT16 = work.tile([128, 4], F16)
        prod = work.tile([128, 128], F32)
        fp = pfused.tile([1, H_DIM], F32)
        for c in range(4):
            nc.vector.scalar_tensor_tensor(
                out=prod[:],
                in0=ctxp[:, c, :],
                scalar=1.0,
                in1=v_T16[:, c * NV : (c + 1) * NV],
                op0=BYPASS,
                op1=MULT,
                accum_out=headT[:, c : c + 1],
            )
            nc.vector.tensor_copy(headT16[:, c : c + 1], headT[:, c : c + 1])
            nc.tensor.matmul(
                fp[:],
                headT16[:, c : c + 1],
                Wo16[:, c * H_DIM : (c + 1) * H_DIM],
                start=(c == 0),
                stop=False,
            )
        nc.tensor.matmul(fp[:], ones16[0:1, 0:1], bo16_r, start=False, stop=True)
        fused_sb = work.tile([1, H_DIM], F32)
        nc.scalar.copy(fused_sb[:], fp[:])
        nc.sync.dma_start(out=fused_d[:], in_=fused_sb[:])

    with tile.TileContext(nc) as tc:
        with (
            tc.tile_pool(name="consts", bufs=1) as consts,
            tc.tile_pool(name="work", bufs=1) as work,
            tc.tile_pool(name="pbig", bufs=3, space="PSUM") as pbig,
            tc.tile_pool(name="pquad", bufs=2, space="PSUM") as pquad,
            tc.tile_pool(name="pfused", bufs=1, space="PSUM") as pfused,
        ):
            emit(consts, work, pbig, pquad, pfused)

    nc.compile()
    return nc


def get_nc(stage=99):
    key = ("nc", stage)
    if key not in _CACHE:
        _CACHE[key] = _build(stage)
    return _CACHE[key]


def prep_weights(Wv, bv, Wq, bq, wa, Wo, bo):
    """Host-side layout prep of the replicated weights (pure layout/dtype)."""
    f16, f32 = np.float16, np.float32
    Wq16 = np.asarray(Wq, f32).astype(f16)
    Wv16 = np.asarray(Wv, f32).astype(f16)
    Wo = np.asarray(Wo, f32)
    Wo16 = (
        np.transpose(Wo.reshape(4, 128, H_DIM), (1, 0, 2))
        .reshape(128, 4 * H_DIM)
        .astype(f16)
    )
    bv_T = np.asarray(bv, f32).reshape(4, 128).T            # [128, 4]
    bq_T = np.asarray(bq, f32).reshape(4, 128).T            # [128, 4]
    wa_T = np.asarray(wa, f32).reshape(2, 128).T            # [128, 2]
    wa_T4 = np.concatenate([wa_T, wa_T], axis=1)            # [128, 4]
    CT32 = np.ascontiguousarray(
        np.concatenate([bv_T, bq_T * wa_T4, wa_T4], axis=1).astype(f32)
    )
    bq16 = np.asarray(bq, f32).reshape(1, HD).astype(f16)
    bo16 = np.asarray(bo, f32).reshape(1, H_DIM).astype(f16)
    rows16 = np.ascontiguousarray(np.concatenate([bq16, bo16], axis=1))
    return {"CT32": CT32, "ROWS16": rows16}, Wq16, Wv16, Wo16


def prep_example(Wq16, Wv16, Wo16, v_b, q_b):
    """Pack per-core fp16 operands (weights replicated + this example's v/q)."""
    f16 = np.float16
    vT16 = np.ascontiguousarray(np.asarray(v_b, np.float32).T).astype(f16)
    qT16 = np.ascontiguousarray(np.asarray(q_b, np.float32).T).astype(f16)
    pk = np.concatenate([Wq16, qT16, Wv16, vT16, Wo16], axis=1)
    assert pk.shape == (128, PK_COLS)
    return {"PK16": np.ascontiguousarray(pk)}


def unprep_att(attT16):
    """[128 j, 8 (h,jc), 128 i] fp16 -> [2, 128 i, 512 j] fp32."""
    a = np.asarray(attT16).reshape(128, 2, 4, 128).astype(np.float32)
    return np.transpose(a, (1, 3, 2, 0)).reshape(H_OUT, NV, NQ)


def make_in_maps(v, q, Wv, bv, Wq, bq, wa, Wo, bo):
    common, Wq16, Wv16, Wo16 = prep_weights(Wv, bv, Wq, bq, wa, Wo, bo)
    B = np.asarray(v).shape[0]
    return [
        dict(common, **prep_example(Wq16, Wv16, Wo16, v[b], q[b])) for b in range(B)
    ]


def kernel(v, q, Wv, bv, Wq, bq, wa, ba, Wo, bo):
    from concourse.bass_utils import run_bass_kernel_spmd

    nc = get_nc()
    B = np.asarray(v).shape[0]
    assert B == N_CORES, f"expected batch {N_CORES}, got {B}"
    in_maps = make_in_maps(v, q, Wv, bv, Wq, bq, wa, Wo, bo)
    res = run_bass_kernel_spmd(nc, in_maps, core_ids=list(range(N_CORES)))
    fused = np.concatenate([r["fused"] for r in res.results], axis=0)
    att = np.stack([unprep_att(r["attT16"]) for r in res.results], axis=0)
    att = att.reshape(B, H_OUT, NV * NQ, 1)
    return fused, att


# revision 23
# speedup vs baseline: 1.8766x; 1.0742x over previous
"""BANLayer Trainium2 kernel.

Data-parallel over batch: 8 examples -> 8 NeuronCores, weights replicated.

The host wrapper does layout-only prep (transposes / fp16 casts / bias-vector
reshapes — no FLOPs on activations beyond dtype rounding); the device does all
the math:

Per-core (one example; i=num_v=128, j=num_q=512, k=in_dim=128,
hd=H_OUT*H_DIM=512; chunk c in 0..3 <-> (h, dhalf)):

  q_T[hd, j]  = Wq.T @ q.T                  (4 MMs, N=512)
  qs_T        = q_T * wa + (bq*wa)          (DVE scale+bias, per-partition, fp16)
  v_T[hd, i]  = Wv.T @ v.T (+bv via ACT)    (4 MMs, N=128)
  S_T[j, i]   = qs_T.T @ v_T  per head      (16 MMs; softmax dim = free dim;
                                             no max-subtraction: |S|<~0.2 and
                                             the +ba shift cancels in softmax)
  E_T, colsum = ACT Exp with accum_out      (8 ACT ops, colsum for free)
  r[j]        = 1/colsum                    (DVE reciprocal, per head)
  E_sc        = E_T * r                     (= att transposed; fp16; DMA'd out
                                             and un-transposed on host)
  q_nat[j,hd] = q @ Wq + bq                 (4+4 MMs; bias via K=1 ones x bq)
  ctx_T[d, i] = q_nat.T @ E_sc              (16 MMs; deferred normalization in E_sc)
  head[hd]    = sum_i v_T * ctx_T           (DVE fused mult + free-dim accum)
  fused       = head @ Wo + bo              (5 fp16 MMs)

fp16 matmul operands, fp32 accumulation/softmax/outputs.

Big fp16 operands are shipped as one packed DRAM tensor to minimize DMA count
(fewer semaphores -> shorter kernel prologue/epilogue).
"""

import numpy as np

H_OUT = 2
H_DIM = 256
NV = 128
NQ = 512
KD = 128          # V_DIM == Q_DIM
HD = H_OUT * H_DIM
N_CORES = 8

# packed fp16 input layout: [128, PK_COLS]
PK_WQ = 0                     # Wq16           [128, 512]
PK_QT = PK_WQ + HD            # qT16           [128, 512]
PK_WV = PK_QT + NQ            # Wv16           [128, 512]
PK_VT = PK_WV + HD            # vT16           [128, 128]
PK_WO = PK_VT + NV            # Wo16 (chunked) [128, 1024]
PK_COLS = PK_WO + 4 * H_DIM   # 2688

_CACHE = {}


def _build(stage=99):
    import concourse.bacc as bacc
    import concourse.tile as tile
    from concourse import mybir

    F32 = mybir.dt.float32
    F16 = mybir.dt.float16
    Identity = mybir.ActivationFunctionType.Identity
    Exp = mybir.ActivationFunctionType.Exp
    MULT = mybir.AluOpType.mult
    ADD = mybir.AluOpType.add
    BYPASS = mybir.AluOpType.bypass

    nc = bacc.Bacc("TRN2", target_bir_lowering=False, debug=False)

    ct_d = nc.dram_tensor("CT32", [128, 12], F32, kind="ExternalInput")
    pk_d = nc.dram_tensor("PK16", [128, PK_COLS], F16, kind="ExternalInput")
    rows_d = nc.dram_tensor("ROWS16", [1, HD + H_DIM], F16, kind="ExternalInput")

    fused_d = nc.dram_tensor("fused", [1, H_DIM], F32, kind="ExternalOutput")
    attT_d = nc.dram_tensor("attT16", [128, 8 * 128], F16, kind="ExternalOutput")

    def emit(consts, work, pbig, pquad, pfused):
        # ---------------- loads (q-side pack first: feeds the first MMs) --------
        pk = work.tile([128, PK_COLS], F16)
        nc.sync.dma_start(out=pk[:, : PK_WV], in_=pk_d[:, : PK_WV])
        CT = work.tile([128, 12], F32)  # 0-3 bv_T, 4-7 (bq*wa)_T, 8-11 wa_T4
        nc.sync.dma_start(out=CT[:], in_=ct_d[:])
        nc.sync.dma_start(out=pk[:, PK_WV :], in_=pk_d[:, PK_WV :])
        rows = work.tile([1, HD + H_DIM], F16)
        nc.sync.dma_start(out=rows[:], in_=rows_d[:])

        Wq16 = pk[:, PK_WQ : PK_WQ + HD]
        qT16 = pk[:, PK_QT : PK_QT + NQ]
        Wv16 = pk[:, PK_WV : PK_WV + HD]
        vT16 = pk[:, PK_VT : PK_VT + NV]
        Wo16 = pk[:, PK_WO : PK_WO + 4 * H_DIM]
        bq16_r = rows[:, :HD]
        bo16_r = rows[:, HD:]

        ones16 = consts.tile([1, 128], F16)
        nc.vector.memset(ones16[:], 1.0)
        zeros1 = consts.tile([128, 1], F32)
        nc.vector.memset(zeros1[:], 0.0)

        # ---------------- q_T -> qs_T ----------------
        qs_T = work.tile([128, 4 * NQ], F16)  # [d, c, j]
        for c in range(4):
            qp = pbig.tile([128, NQ], F32, tag="big")
            nc.tensor.matmul(qp[:], Wq16[:, c * 128 : (c + 1) * 128], qT16)
            nc.vector.tensor_scalar(
                out=qs_T[:, c * NQ : (c + 1) * NQ],
                in0=qp[:],
                scalar1=CT[:, 8 + c : 9 + c],
                scalar2=CT[:, 4 + c : 5 + c],
                op0=MULT,
                op1=ADD,
            )

        # ---------------- q_nat (independent of softmax path) ----------------
        q_nat16 = work.tile([128, 4 * HD], F16)  # [j, jc, hd]
        for jc in range(4):
            qnp = pbig.tile([128, HD], F32, tag="big")
            nc.tensor.matmul(
                qnp[:], qT16[:, jc * 128 : (jc + 1) * 128], Wq16, start=True, stop=False
            )
            nc.tensor.matmul(qnp[:], ones16[:], bq16_r, start=False, stop=True)
            # split psum->SBUF copies across ACT and DVE for engine balance
            if jc % 2 == 0:
                nc.scalar.copy(q_nat16[:, jc * HD : (jc + 1) * HD], qnp[:])
            else:
                nc.vector.tensor_copy(q_nat16[:, jc * HD : (jc + 1) * HD], qnp[:])

        # ---------------- v_T ----------------
        v_T16 = work.tile([128, 4 * NV], F16)  # [d, c, i]
        vp = pquad.tile([128, 4, 128], F32, tag="quad")
        for c in range(4):
            nc.tensor.matmul(vp[:, c, :], Wv16[:, c * 128 : (c + 1) * 128], vT16)
        for c in range(4):
            nc.scalar.activation(
                v_T16[:, c * NV : (c + 1) * NV],
                vp[:, c, :],
                Identity,
                bias=CT[:, c : c + 1],
                scale=1.0,
            )

        if stage <= 1:
            return

        # ---------------- score + softmax (transposed layout) ----------------
        E_T16 = work.tile([128, 8 * 128], F16)  # [j, (h,jc), i]
        E_sc16 = work.tile([128, 8 * 128], F16)
        colsT = work.tile([128, 8], F32)
        r_T = work.tile([128, 8], F32)
        AXX = mybir.AxisListType.X
        for h in range(2):
            sp = pquad.tile([128, NQ], F32, tag="quad")
            for jc in range(4):
                for dc in range(2):
                    c = h * 2 + dc
                    nc.tensor.matmul(
                        sp[:, jc * 128 : (jc + 1) * 128],
                        qs_T[:, c * NQ + jc * 128 : c * NQ + (jc + 1) * 128],
                        v_T16[:, c * NV : (c + 1) * NV],
                        start=(dc == 0),
                        stop=(dc == 1),
                    )
            # one big exp per head; colsum via one 3D free-dim reduce
            nc.scalar.activation(
                E_T16[:, h * 512 : (h + 1) * 512], sp[:], Exp, bias=zeros1[:]
            )
            nc.vector.reduce_sum(
                out=colsT[:, h * 4 : (h + 1) * 4],
                in_=E_T16[:, h * 512 : (h + 1) * 512].rearrange(
                    "p (a b) -> p a b", b=128
                ),
                axis=AXX,
            )
            nc.vector.reciprocal(
                r_T[:, h * 4 : (h + 1) * 4], colsT[:, h * 4 : (h + 1) * 4]
            )
            for jc in range(4):
                hc = h * 4 + jc
                if jc % 2 == 0:
                    nc.vector.tensor_scalar(
                        out=E_sc16[:, hc * 128 : (hc + 1) * 128],
                        in0=E_T16[:, hc * 128 : (hc + 1) * 128],
                        scalar1=r_T[:, hc : hc + 1],
                        scalar2=None,
                        op0=MULT,
                    )
                else:
                    nc.scalar.activation(
                        E_sc16[:, hc * 128 : (hc + 1) * 128],
                        E_T16[:, hc * 128 : (hc + 1) * 128],
                        Identity,
                        bias=zeros1[:],
                        scale=r_T[:, hc : hc + 1],
                    )
            nc.sync.dma_start(
                out=attT_d[:, h * 512 : (h + 1) * 512],
                in_=E_sc16[:, h * 512 : (h + 1) * 512],
            )

        if stage <= 2:
            return

        # ---------------- ctx (transposed) + head + fused ----------------
        ctxp = pquad.tile([128, 4, 128], F32, tag="quad")
        for h in range(2):
            for dh in range(2):
                c = h * 2 + dh
                for jc in range(4):
                    nc.tensor.matmul(
                        ctxp[:, c, :],
                        q_nat16[:, jc * HD + c * 128 : jc * HD + (c + 1) * 128],
                        E_sc16[:, (h * 4 + jc) * 128 : (h * 4 + jc + 1) * 128],
                        start=(jc == 0),
                        stop=(jc == 3),
                    )
        headT = work.tile([128, 4], F32)
        headT16 = work.tile([128, 4], F16)
        prod = work.tile([128, 128], F32)
        fp = pfused.tile([1, H_DIM], F32)
        for c in range(4):
            nc.vector.scalar_tensor_tensor(
                out=prod[:],
                in0=ctxp[:, c, :],
                scalar=1.0,
                in1=v_T16[:, c * NV : (c + 1) * NV],
                op0=BYPASS,
                op1=MULT,
                accum_out=headT[:, c : c + 1],
            )
            nc.vector.tensor_copy(headT16[:, c : c + 1], headT[:, c : c + 1])
            nc.tensor.matmul(
                fp[:],
                headT16[:, c : c + 1],
                Wo16[:, c * H_DIM : (c + 1) * H_DIM],
                start=(c == 0),
                stop=False,
            )
        nc.tensor.matmul(fp[:], ones16[0:1, 0:1], bo16_r, start=False, stop=True)
        fused_sb = work.tile([1, H_DIM], F32)
        nc.scalar.copy(fused_sb[:], fp[:])
        nc.sync.dma_start(out=fused_d[:], in_=fused_sb[:])

    with tile.TileContext(nc) as tc:
        with (
            tc.tile_pool(name="consts", bufs=1) as consts,
            tc.tile_pool(name="work", bufs=1) as work,
            tc.tile_pool(name="pbig", bufs=3, space="PSUM") as pbig,
            tc.tile_pool(name="pquad", bufs=2, space="PSUM") as pquad,
            tc.tile_pool(name="pfused", bufs=1, space="PSUM") as pfused,
        ):
            emit(consts, work, pbig, pquad, pfused)

    nc.compile()
    return nc


def get_nc(stage=99):
    key = ("nc", stage)
    if key not in _CACHE:
        _CACHE[key] = _build(stage)
    return _CACHE[key]


def prep_weights(Wv, bv, Wq, bq, wa, Wo, bo):
    """Host-side layout prep of the replicated weights (pure layout/dtype)."""
    f16, f32 = np.float16, np.float32
    Wq16 = np.asarray(Wq, f32).astype(f16)
    Wv16 = np.asarray(Wv, f32).astype(f16)
    Wo = np.asarray(Wo, f32)
    Wo16 = (
        np.transpose(Wo.reshape(4, 128, H_DIM), (1, 0, 2))
        .reshape(128, 4 * H_DIM)
        .astype(f16)
    )
    bv_T = np.asarray(bv, f32).reshape(4, 128).T            # [128, 4]
    bq_T = np.asarray(bq, f32).reshape(4, 128).T            # [128, 4]
    wa_T = np.asarray(wa, f32).reshape(2, 128).T            # [128, 2]
    wa_T4 = np.concatenate([wa_T, wa_T], axis=1)            # [128, 4]
    CT32 = np.ascontiguousarray(
        np.concatenate([bv_T, bq_T * wa_T4, wa_T4], axis=1).astype(f32)
    )
    bq16 = np.asarray(bq, f32).reshape(1, HD).astype(f16)
    bo16 = np.asarray(bo, f32).reshape(1, H_DIM).astype(f16)
    rows16 = np.ascontiguousarray(np.concatenate([bq16, bo16], axis=1))
    return {"CT32": CT32, "ROWS16": rows16}, Wq16, Wv16, Wo16


def prep_example(Wq16, Wv16, Wo16, v_b, q_b):
    """Pack per-core fp16 operands (weights replicated + this example's v/q)."""
    f16 = np.float16
    vT16 = np.ascontiguousarray(np.asarray(v_b, np.float32).T).astype(f16)
    qT16 = np.ascontiguousarray(np.asarray(q_b, np.float32).T).astype(f16)
    pk = np.concatenate([Wq16, qT16, Wv16, vT16, Wo16], axis=1)
    assert pk.shape == (128, PK_COLS)
    return {"PK16": np.ascontiguousarray(pk)}


def unprep_att(attT16):
    """[128 j, 8 (h,jc), 128 i] fp16 -> [2, 128 i, 512 j] fp32."""
    a = np.asarray(attT16).reshape(128, 2, 4, 128).astype(np.float32)
    return np.transpose(a, (1, 3, 2, 0)).reshape(H_OUT, NV, NQ)


def make_in_maps(v, q, Wv, bv, Wq, bq, wa, Wo, bo):
    common, Wq16, Wv16, Wo16 = prep_weights(Wv, bv, Wq, bq, wa, Wo, bo)
    B = np.asarray(v).shape[0]
    return [
        dict(common, **prep_example(Wq16, Wv16, Wo16, v[b], q[b])) for b in range(B)
    ]


def kernel(v, q, Wv, bv, Wq, bq, wa, ba, Wo, bo):
    from concourse.bass_utils import run_bass_kernel_spmd

    nc = get_nc()
    B = np.asarray(v).shape[0]
    assert B == N_CORES, f"expected batch {N_CORES}, got {B}"
    in_maps = make_in_maps(v, q, Wv, bv, Wq, bq, wa, Wo, bo)
    res = run_bass_kernel_spmd(nc, in_maps, core_ids=list(range(N_CORES)))
    fused = np.concatenate([r["fused"] for r in res.results], axis=0)
    att = np.stack([unprep_att(r["attT16"]) for r in res.results], axis=0)
    att = att.reshape(B, H_OUT, NV * NQ, 1)
    return fused, att


# revision 25
# speedup vs baseline: 1.9401x; 1.0338x over previous
"""BANLayer Trainium2 kernel.

Data-parallel over batch: 8 examples -> 8 NeuronCores, weights replicated.

The host wrapper does layout-only prep (transposes / fp16 casts / bias-vector
reshapes/broadcasts — no FLOPs on activations beyond dtype rounding); the
device does all the math:

Per-core (one example; i=num_v=128, j=num_q=512, k=in_dim=128,
hd=H_OUT*H_DIM=512; chunk c in 0..3 <-> (h, dhalf)):

  q_T[hd, j]  = Wq.T @ q.T                  (4 MMs, N=512)
  qs_T        = q_T * wa + (bq*wa)          (DVE scale+bias, per-partition, fp16)
  v_T[hd, i]  = Wv.T @ v.T (+bv via ACT)    (4 MMs, N=128)
  S_T[j, i]   = qs_T.T @ v_T  per head      (16 MMs; softmax dim = free dim;
                                             no max-subtraction: |S|<~0.2 and
                                             the +ba shift cancels in softmax)
  E_T         = exp(S_T)                    (1 big ACT op per head)
  colsum, r   = DVE 3D reduce + reciprocal  (per head)
  E_sc        = E_T * r                     (= att transposed; fp16; DMA'd out
                                             per head, un-transposed on host)
  q_nat[j,hd] = q @ Wq + bq                 (4 MMs; bias added during psum
                                             evacuation vs a host-broadcast bq)
  ctx_T[d, i] = q_nat.T @ E_sc              (16 MMs; deferred normalization in E_sc)
  head[hd]    = sum_i v_T * ctx_T           (DVE fused mult + free-dim accum)
  fused       = head @ Wo + bo              (5 fp16 MMs, interleaved with ctx)

fp16 matmul operands, fp32 accumulation/softmax/outputs.

Big fp16 operands ship as one packed DRAM tensor (fewer DMAs -> fewer
semaphores); loads are split across the sync and scalar HWDGE queues.
"""

import numpy as np

H_OUT = 2
H_DIM = 256
NV = 128
NQ = 512
KD = 128          # V_DIM == Q_DIM
HD = H_OUT * H_DIM
N_CORES = 8

# packed fp16 input layout: [128, PK_COLS]
PK_WQ = 0                     # Wq16            [128, 512]
PK_QT = PK_WQ + HD            # qT16            [128, 512]
PK_WV = PK_QT + NQ            # Wv16            [128, 512]
PK_VT = PK_WV + HD            # vT16            [128, 128]
PK_WO = PK_VT + NV            # Wo16 (chunked)  [128, 1024]
PK_BQ = PK_WO + 4 * H_DIM     # bq16 broadcast  [128, 512]
PK_COLS = PK_BQ + HD          # 3200

_CACHE = {}


def _build(stage=99):
    import concourse.bacc as bacc
    import concourse.tile as tile
    from concourse import mybir

    F32 = mybir.dt.float32
    F16 = mybir.dt.float16
    Identity = mybir.ActivationFunctionType.Identity
    Exp = mybir.ActivationFunctionType.Exp
    MULT = mybir.AluOpType.mult
    ADD = mybir.AluOpType.add
    BYPASS = mybir.AluOpType.bypass

    nc = bacc.Bacc("TRN2", target_bir_lowering=False, debug=False)

    ct_d = nc.dram_tensor("CT32", [128, 12], F32, kind="ExternalInput")
    pk_d = nc.dram_tensor("PK16", [128, PK_COLS], F16, kind="ExternalInput")
    rows_d = nc.dram_tensor("ROWS16", [1, H_DIM], F16, kind="ExternalInput")

    fused_d = nc.dram_tensor("fused", [1, H_DIM], F32, kind="ExternalOutput")
    attT_d = nc.dram_tensor("attT16", [128, 8 * 128], F16, kind="ExternalOutput")

    def emit(consts, work, pbig, pquad, pfused):
        # ------------- loads (q-side pack first; small ones on scalar queue) ----
        pk = work.tile([128, PK_COLS], F16)
        nc.sync.dma_start(out=pk[:, : PK_WV], in_=pk_d[:, : PK_WV])
        CT = work.tile([128, 12], F32)  # 0-3 bv_T, 4-7 (bq*wa)_T, 8-11 wa_T4
        nc.scalar.dma_start(out=CT[:], in_=ct_d[:])
        rows = work.tile([1, H_DIM], F16)
        nc.scalar.dma_start(out=rows[:], in_=rows_d[:])
        nc.sync.dma_start(out=pk[:, PK_WV :], in_=pk_d[:, PK_WV :])

        Wq16 = pk[:, PK_WQ : PK_WQ + HD]
        qT16 = pk[:, PK_QT : PK_QT + NQ]
        Wv16 = pk[:, PK_WV : PK_WV + HD]
        vT16 = pk[:, PK_VT : PK_VT + NV]
        Wo16 = pk[:, PK_WO : PK_WO + 4 * H_DIM]
        bqb16 = pk[:, PK_BQ : PK_BQ + HD]
        bo16_r = rows[:, :]

        ones16 = consts.tile([1, 128], F16)
        nc.vector.memset(ones16[:], 1.0)
        zeros1 = consts.tile([128, 1], F32)
        nc.vector.memset(zeros1[:], 0.0)

        # ---------------- q_T -> qs_T ----------------
        qs_T = work.tile([128, 4 * NQ], F16)  # [d, c, j]
        for c in range(4):
            qp = pbig.tile([128, NQ], F32, tag="big")
            nc.tensor.matmul(qp[:], Wq16[:, c * 128 : (c + 1) * 128], qT16)
            nc.vector.tensor_scalar(
                out=qs_T[:, c * NQ : (c + 1) * NQ],
                in0=qp[:],
                scalar1=CT[:, 8 + c : 9 + c],
                scalar2=CT[:, 4 + c : 5 + c],
                op0=MULT,
                op1=ADD,
            )

        # ---------------- q_nat matmuls (PE filler; evacuation comes later) ----
        q_nat16 = work.tile([128, 4 * HD], F16)  # [j, jc, hd]
        qnps = []
        for jc in range(4):
            qnp = pbig.tile([128, HD], F32, tag="big")
            nc.tensor.matmul(qnp[:], qT16[:, jc * 128 : (jc + 1) * 128], Wq16)
            qnps.append(qnp)

        # ---------------- v_T ----------------
        v_T16 = work.tile([128, 4 * NV], F16)  # [d, c, i]
        vp = pquad.tile([128, NQ], F32, tag="quad")
        for c in range(4):
            nc.tensor.matmul(vp[:, c * 128 : (c + 1) * 128], Wv16[:, c * 128 : (c + 1) * 128], vT16)
        for c in range(4):
            nc.scalar.activation(
                v_T16[:, c * NV : (c + 1) * NV],
                vp[:, c * 128 : (c + 1) * 128],
                Identity,
                bias=CT[:, c : c + 1],
                scale=1.0,
            )

        if stage <= 1:
            return

        # ---------------- score + softmax (transposed layout) ----------------
        E_T16 = work.tile([128, 8 * 128], F16)  # [j, (h,jc), i]
        E_sc16 = work.tile([128, 8 * 128], F16)
        colsT = work.tile([128, 8], F32)
        r_T = work.tile([128, 8], F32)
        AXX = mybir.AxisListType.X
        for h in range(2):
            sp = pquad.tile([128, NQ], F32, tag="quad")
            for jc in range(4):
                for dc in range(2):
                    c = h * 2 + dc
                    nc.tensor.matmul(
                        sp[:, jc * 128 : (jc + 1) * 128],
                        qs_T[:, c * NQ + jc * 128 : c * NQ + (jc + 1) * 128],
                        v_T16[:, c * NV : (c + 1) * NV],
                        start=(dc == 0),
                        stop=(dc == 1),
                    )
            # one big exp per head; colsum via one 3D free-dim reduce
            nc.scalar.activation(
                E_T16[:, h * 512 : (h + 1) * 512], sp[:], Exp, bias=zeros1[:]
            )
            nc.vector.reduce_sum(
                out=colsT[:, h * 4 : (h + 1) * 4],
                in_=E_T16[:, h * 512 : (h + 1) * 512].rearrange(
                    "p (a b) -> p a b", b=128
                ),
                axis=AXX,
            )
            nc.vector.reciprocal(
                r_T[:, h * 4 : (h + 1) * 4], colsT[:, h * 4 : (h + 1) * 4]
            )
            for jc in range(4):
                hc = h * 4 + jc
                if jc % 2 == 0:
                    nc.vector.tensor_scalar(
                        out=E_sc16[:, hc * 128 : (hc + 1) * 128],
                        in0=E_T16[:, hc * 128 : (hc + 1) * 128],
                        scalar1=r_T[:, hc : hc + 1],
                        scalar2=None,
                        op0=MULT,
                    )
                else:
                    nc.scalar.activation(
                        E_sc16[:, hc * 128 : (hc + 1) * 128],
                        E_T16[:, hc * 128 : (hc + 1) * 128],
                        Identity,
                        bias=zeros1[:],
                        scale=r_T[:, hc : hc + 1],
                    )
            nc.sync.dma_start(
                out=attT_d[:, h * 512 : (h + 1) * 512],
                in_=E_sc16[:, h * 512 : (h + 1) * 512],
            )
            if h == 0:
                # evacuate q_nat psum (+ bias add vs host-broadcast bq) off the
                # critical softmax path: emitted after head 0's chain
                for jc in range(4):
                    nc.vector.tensor_tensor(
                        out=q_nat16[:, jc * HD : (jc + 1) * HD],
                        in0=qnps[jc][:],
                        in1=bqb16,
                        op=ADD,
                    )

        if stage <= 2:
            return

        # ---------------- ctx (transposed) + head + fused (interleaved) -------
        ctxp = pquad.tile([128, NQ], F32, tag="quad")
        headT = work.tile([128, 4], F32)
        headT16 = work.tile([128, 4], F16)
        prod = work.tile([128, 128], F32)
        fp = pfused.tile([1, H_DIM], F32)
        for h in range(2):
            for dh in range(2):
                c = h * 2 + dh
                for jc in range(4):
                    nc.tensor.matmul(
                        ctxp[:, c * 128 : (c + 1) * 128],
                        q_nat16[:, jc * HD + c * 128 : jc * HD + (c + 1) * 128],
                        E_sc16[:, (h * 4 + jc) * 128 : (h * 4 + jc + 1) * 128],
                        start=(jc == 0),
                        stop=(jc == 3),
                    )
                nc.vector.scalar_tensor_tensor(
                    out=prod[:],
                    in0=ctxp[:, c * 128 : (c + 1) * 128],
                    scalar=1.0,
                    in1=v_T16[:, c * NV : (c + 1) * NV],
                    op0=BYPASS,
                    op1=MULT,
                    accum_out=headT[:, c : c + 1],
                )
                nc.vector.tensor_copy(headT16[:, c : c + 1], headT[:, c : c + 1])
                nc.tensor.matmul(
                    fp[:],
                    headT16[:, c : c + 1],
                    Wo16[:, c * H_DIM : (c + 1) * H_DIM],
                    start=(c == 0),
                    stop=False,
                )
        nc.tensor.matmul(fp[:], ones16[0:1, 0:1], bo16_r, start=False, stop=True)
        fused_sb = work.tile([1, H_DIM], F32)
        nc.scalar.copy(fused_sb[:], fp[:])
        nc.sync.dma_start(out=fused_d[:], in_=fused_sb[:])

    with tile.TileContext(nc) as tc:
        with (
            tc.tile_pool(name="consts", bufs=1) as consts,
            tc.tile_pool(name="work", bufs=1) as work,
            tc.tile_pool(name="pbig", bufs=5, space="PSUM") as pbig,
            tc.tile_pool(name="pquad", bufs=2, space="PSUM") as pquad,
            tc.tile_pool(name="pfused", bufs=1, space="PSUM") as pfused,
        ):
            emit(consts, work, pbig, pquad, pfused)

    nc.compile()
    return nc


def get_nc(stage=99):
    key = ("nc", stage)
    if key not in _CACHE:
        _CACHE[key] = _build(stage)
    return _CACHE[key]


def prep_weights(Wv, bv, Wq, bq, wa, Wo, bo):
    """Host-side layout prep of the replicated weights (pure layout/dtype)."""
    f16, f32 = np.float16, np.float32
    Wq16 = np.asarray(Wq, f32).astype(f16)
    Wv16 = np.asarray(Wv, f32).astype(f16)
    Wo = np.asarray(Wo, f32)
    Wo16 = (
        np.transpose(Wo.reshape(4, 128, H_DIM), (1, 0, 2))
        .reshape(128, 4 * H_DIM)
        .astype(f16)
    )
    bq16 = np.asarray(bq, f32).reshape(1, HD).astype(f16)
    bqb16 = np.broadcast_to(bq16, (128, HD))
    bv_T = np.asarray(bv, f32).reshape(4, 128).T            # [128, 4]
    bq_T = np.asarray(bq, f32).reshape(4, 128).T            # [128, 4]
    wa_T = np.asarray(wa, f32).reshape(2, 128).T            # [128, 2]
    wa_T4 = np.concatenate([wa_T, wa_T], axis=1)            # [128, 4]
    CT32 = np.ascontiguousarray(
        np.concatenate([bv_T, bq_T * wa_T4, wa_T4], axis=1).astype(f32)
    )
    bo16 = np.asarray(bo, f32).reshape(1, H_DIM).astype(f16)
    return {"CT32": CT32, "ROWS16": bo16}, Wq16, Wv16, Wo16, bqb16


def prep_example(Wq16, Wv16, Wo16, bqb16, v_b, q_b):
    """Pack per-core fp16 operands (weights replicated + this example's v/q)."""
    f16 = np.float16
    vT16 = np.ascontiguousarray(np.asarray(v_b, np.float32).T).astype(f16)
    qT16 = np.ascontiguousarray(np.asarray(q_b, np.float32).T).astype(f16)
    pk = np.concatenate([Wq16, qT16, Wv16, vT16, Wo16, bqb16], axis=1)
    assert pk.shape == (128, PK_COLS)
    return {"PK16": np.ascontiguousarray(pk)}


def unprep_att(attT16):
    """[128 j, 8 (h,jc), 128 i] fp16 -> [2, 128 i, 512 j] fp32."""
    a = np.asarray(attT16).reshape(128, 2, 4, 128).astype(np.float32)
    return np.transpose(a, (1, 3, 2, 0)).reshape(H_OUT, NV, NQ)


def make_in_maps(v, q, Wv, bv, Wq, bq, wa, Wo, bo):
    common, Wq16, Wv16, Wo16, bqb16 = prep_weights(Wv, bv, Wq, bq, wa, Wo, bo)
    B = np.asarray(v).shape[0]
    return [
        dict(common, **prep_example(Wq16, Wv16, Wo16, bqb16, v[b], q[b]))
        for b in range(B)
    ]


def kernel(v, q, Wv, bv, Wq, bq, wa, ba, Wo, bo):
    from concourse.bass_utils import run_bass_kernel_spmd

    nc = get_nc()
    B = np.asarray(v).shape[0]
    assert B == N_CORES, f"expected batch {N_CORES}, got {B}"
    in_maps = make_in_maps(v, q, Wv, bv, Wq, bq, wa, Wo, bo)
    res = run_bass_kernel_spmd(nc, in_maps, core_ids=list(range(N_CORES)))
    fused = np.concatenate([r["fused"] for r in res.results], axis=0)
    att = np.stack([unprep_att(r["attT16"]) for r in res.results], axis=0)
    att = att.reshape(B, H_OUT, NV * NQ, 1)
    return fused, att


# revision 28
# speedup vs baseline: 2.0642x; 1.0640x over previous
"""BANLayer Trainium2 kernel.

Data-parallel over batch: 8 examples -> 8 NeuronCores, weights replicated.

The host wrapper does layout-only prep (transposes / fp16 casts / bias-vector
reshapes/broadcasts — no FLOPs on activations beyond dtype rounding); the
device does all the math:

Per-core (one example; i=num_v=128, j=num_q=512, k=in_dim=128,
hd=H_OUT*H_DIM=512; chunk c in 0..3 <-> (h, dhalf)):

  v_T[hd, i]  = Wv.T @ v.T (+bv)            (4 MMs, N=128)
  q_T[hd, j]  = Wq.T @ q.T                  (4 MMs, N=512)
  qs_T        = q_T * wa + (bq*wa)          (per-partition scale+bias, fp16)
  S_T[j, i]   = qs_T.T @ v_T  per head      (16 MMs; softmax dim = free dim;
                                             no max-subtraction: |S|<~0.2 and
                                             the +ba shift cancels in softmax)
  E_T         = exp(S_T)                    (1 big ACT op per head)
  colsum, r   = DVE 3D reduce + reciprocal  (per head)
  E_sc        = E_T * r                     (= att transposed; fp16; DMA'd out
                                             per head, un-transposed on host)
  q_nat[j,hd] = q @ Wq + bq                 (4 MMs, scheduled into the softmax
                                             latency; bias added during psum
                                             evacuation vs a host-broadcast bq)
  ctx_T[d, i] = q_nat.T @ E_sc              (16 MMs; deferred normalization in E_sc)
  head[hd]    = sum_i v_T * ctx_T           (DVE fused mult + free-dim accum)
  fusedT[o]   = Wo.T @ head (+bo)           (8 tiny N=1 MMs; output transposed,
                                             un-transposed on host)

fp16 matmul operands, fp32 accumulation/softmax/outputs.
"""

import numpy as np

H_OUT = 2
H_DIM = 256
NV = 128
NQ = 512
KD = 128          # V_DIM == Q_DIM
HD = H_OUT * H_DIM
N_CORES = 8

# packed fp16 input layout: [128, PK_COLS]
PK_WV = 0                     # Wv16            [128, 512]
PK_VT = PK_WV + HD            # vT16            [128, 128]
PK_WQ = PK_VT + NV            # Wq16            [128, 512]
PK_QT = PK_WQ + HD            # qT16            [128, 512]
PK_WO = PK_QT + NQ            # Wo16 (chunked)  [128, 1024]
PK_BQ = PK_WO + 4 * H_DIM     # bq16 broadcast  [128, 512]
PK_COLS = PK_BQ + HD          # 3200
PK_A1 = PK_WQ                 # first DMA: v-side
PK_A2 = PK_WO                 # second DMA: q-side

_CACHE = {}


def _build(stage=99):
    import concourse.bacc as bacc
    import concourse.tile as tile
    from concourse import mybir

    F32 = mybir.dt.float32
    F16 = mybir.dt.float16
    Identity = mybir.ActivationFunctionType.Identity
    Exp = mybir.ActivationFunctionType.Exp
    MULT = mybir.AluOpType.mult
    ADD = mybir.AluOpType.add
    BYPASS = mybir.AluOpType.bypass

    nc = bacc.Bacc("TRN2", target_bir_lowering=False, debug=False)

    # CT32 cols: 0-3 bv_T, 4-7 (bq*wa)_T, 8-11 wa_T4, 12-13 bo_T
    ct_d = nc.dram_tensor("CT32", [128, 14], F32, kind="ExternalInput")
    pk_d = nc.dram_tensor("PK16", [128, PK_COLS], F16, kind="ExternalInput")

    fusedT_d = nc.dram_tensor("fusedT", [128, 2], F32, kind="ExternalOutput")
    attT_d = nc.dram_tensor("attT16", [128, 8 * 128], F16, kind="ExternalOutput")

    def emit(consts, work, pbig, pquad, pfused):
        # ------------- loads (v-side first, then q-side, then the rest) --------
        pk = work.tile([128, PK_COLS], F16)
        nc.sync.dma_start(out=pk[:, :PK_A1], in_=pk_d[:, :PK_A1])
        CT = work.tile([128, 14], F32)
        nc.scalar.dma_start(out=CT[:], in_=ct_d[:])
        nc.sync.dma_start(out=pk[:, PK_A1:PK_A2], in_=pk_d[:, PK_A1:PK_A2])
        nc.sync.dma_start(out=pk[:, PK_A2:], in_=pk_d[:, PK_A2:])

        Wv16 = pk[:, PK_WV : PK_WV + HD]
        vT16 = pk[:, PK_VT : PK_VT + NV]
        Wq16 = pk[:, PK_WQ : PK_WQ + HD]
        qT16 = pk[:, PK_QT : PK_QT + NQ]
        Wo16 = pk[:, PK_WO : PK_WO + 4 * H_DIM]
        bqb16 = pk[:, PK_BQ : PK_BQ + HD]

        zeros1 = consts.tile([128, 1], F32)
        nc.vector.memset(zeros1[:], 0.0)

        # ---------------- v_T (first PE work: v-side pack lands first) ---------
        v_T16 = work.tile([128, 4 * NV], F16)  # [d, c, i]
        vp = pquad.tile([128, NQ], F32, tag="quad")
        for c in range(4):
            nc.tensor.matmul(
                vp[:, c * 128 : (c + 1) * 128], Wv16[:, c * 128 : (c + 1) * 128], vT16
            )
        for c in range(4):
            nc.vector.tensor_scalar(
                out=v_T16[:, c * NV : (c + 1) * NV],
                in0=vp[:, c * 128 : (c + 1) * 128],
                scalar1=CT[:, c : c + 1],
                scalar2=None,
                op0=ADD,
            )

        # ---------------- q_T -> qs_T (scale+bias split across DVE/ACT) -------
        qs_T = work.tile([128, 4 * NQ], F16)  # [d, c, j]
        for c in range(4):
            qp = pbig.tile([128, NQ], F32, tag="big")
            nc.tensor.matmul(qp[:], Wq16[:, c * 128 : (c + 1) * 128], qT16)
            if c % 2 == 0:
                nc.vector.tensor_scalar(
                    out=qs_T[:, c * NQ : (c + 1) * NQ],
                    in0=qp[:],
                    scalar1=CT[:, 8 + c : 9 + c],
                    scalar2=CT[:, 4 + c : 5 + c],
                    op0=MULT,
                    op1=ADD,
                )
            else:
                nc.scalar.activation(
                    qs_T[:, c * NQ : (c + 1) * NQ],
                    qp[:],
                    Identity,
                    bias=CT[:, 4 + c : 5 + c],
                    scale=CT[:, 8 + c : 9 + c],
                )

        if stage <= 1:
            return

        # ---------------- score head 0 ----------------
        E_T16 = work.tile([128, 8 * 128], F16)  # [j, (h,jc), i]
        E_sc16 = work.tile([128, 8 * 128], F16)
        colsT = work.tile([128, 8], F32)
        r_T = work.tile([128, 8], F32)
        AXX = mybir.AxisListType.X
        sps = []

        def score_mms(h):
            sp = pquad.tile([128, NQ], F32, tag="quad")
            sps.append(sp)
            for jc in range(4):
                for dc in range(2):
                    c = h * 2 + dc
                    nc.tensor.matmul(
                        sp[:, jc * 128 : (jc + 1) * 128],
                        qs_T[:, c * NQ + jc * 128 : c * NQ + (jc + 1) * 128],
                        v_T16[:, c * NV : (c + 1) * NV],
                        start=(dc == 0),
                        stop=(dc == 1),
                    )

        def softmax(h):
            sp = sps[h]
            nc.scalar.activation(
                E_T16[:, h * 512 : (h + 1) * 512], sp[:], Exp, bias=zeros1[:]
            )
            nc.vector.reduce_sum(
                out=colsT[:, h * 4 : (h + 1) * 4],
                in_=E_T16[:, h * 512 : (h + 1) * 512].rearrange(
                    "p (a b) -> p a b", b=128
                ),
                axis=AXX,
            )
            nc.vector.reciprocal(
                r_T[:, h * 4 : (h + 1) * 4], colsT[:, h * 4 : (h + 1) * 4]
            )
            for jc in range(4):
                hc = h * 4 + jc
                if jc % 2 == 0:
                    nc.vector.tensor_scalar(
                        out=E_sc16[:, hc * 128 : (hc + 1) * 128],
                        in0=E_T16[:, hc * 128 : (hc + 1) * 128],
                        scalar1=r_T[:, hc : hc + 1],
                        scalar2=None,
                        op0=MULT,
                    )
                else:
                    nc.scalar.activation(
                        E_sc16[:, hc * 128 : (hc + 1) * 128],
                        E_T16[:, hc * 128 : (hc + 1) * 128],
                        Identity,
                        bias=zeros1[:],
                        scale=r_T[:, hc : hc + 1],
                    )
            nc.sync.dma_start(
                out=attT_d[:, h * 512 : (h + 1) * 512],
                in_=E_sc16[:, h * 512 : (h + 1) * 512],
            )

        score_mms(0)
        softmax(0)

        # ---------------- q_nat: PE filler during head-0 softmax latency ------
        q_nat16 = work.tile([128, 4 * HD], F16)  # [j, jc, hd]
        for jc in range(4):
            qnp = pbig.tile([128, HD], F32, tag="big")
            nc.tensor.matmul(qnp[:], qT16[:, jc * 128 : (jc + 1) * 128], Wq16)
            nc.vector.tensor_tensor(
                out=q_nat16[:, jc * HD : (jc + 1) * HD],
                in0=qnp[:],
                in1=bqb16,
                op=ADD,
            )

        score_mms(1)
        softmax(1)

        if stage <= 2:
            return

        # ---------------- ctx (transposed) + head + fusedT (interleaved) ------
        ctxp = pquad.tile([128, NQ], F32, tag="quad")
        headT = work.tile([128, 4], F32)
        headT16 = work.tile([128, 4], F16)
        prod = work.tile([128, 128], F32)
        fp = pfused.tile([128, 2], F32)

        def ctx_mms(c):
            h = c // 2
            for jc in range(4):
                nc.tensor.matmul(
                    ctxp[:, c * 128 : (c + 1) * 128],
                    q_nat16[:, jc * HD + c * 128 : jc * HD + (c + 1) * 128],
                    E_sc16[:, (h * 4 + jc) * 128 : (h * 4 + jc + 1) * 128],
                    start=(jc == 0),
                    stop=(jc == 3),
                )

        def head_chunk(c):
            nc.vector.scalar_tensor_tensor(
                out=prod[:],
                in0=ctxp[:, c * 128 : (c + 1) * 128],
                scalar=1.0,
                in1=v_T16[:, c * NV : (c + 1) * NV],
                op0=BYPASS,
                op1=MULT,
                accum_out=headT[:, c : c + 1],
            )
            nc.vector.tensor_copy(headT16[:, c : c + 1], headT[:, c : c + 1])

        def fused_mms(oh):
            # fusedT[oh*128+o, 1] = sum_c Wo[c-chunk, oh-half].T @ headT16[:, c]
            for c in range(4):
                nc.tensor.matmul(
                    fp[:, oh : oh + 1],
                    Wo16[:, c * H_DIM + oh * 128 : c * H_DIM + (oh + 1) * 128],
                    headT16[:, c : c + 1],
                    start=(c == 0),
                    stop=(c == 3),
                )

        ctx_mms(0)
        ctx_mms(1)
        head_chunk(0)
        ctx_mms(2)
        head_chunk(1)
        ctx_mms(3)
        head_chunk(2)
        head_chunk(3)
        fused_mms(0)
        fused_mms(1)

        fused_sb = work.tile([128, 2], F32)
        nc.vector.tensor_tensor(out=fused_sb[:], in0=fp[:], in1=CT[:, 12:14], op=ADD)
        nc.sync.dma_start(out=fusedT_d[:], in_=fused_sb[:])

    with tile.TileContext(nc) as tc:
        with (
            tc.tile_pool(name="consts", bufs=1) as consts,
            tc.tile_pool(name="work", bufs=1) as work,
            tc.tile_pool(name="pbig", bufs=5, space="PSUM") as pbig,
            tc.tile_pool(name="pquad", bufs=2, space="PSUM") as pquad,
            tc.tile_pool(name="pfused", bufs=1, space="PSUM") as pfused,
        ):
            emit(consts, work, pbig, pquad, pfused)

    nc.compile()
    return nc


def get_nc(stage=99):
    key = ("nc", stage)
    if key not in _CACHE:
        _CACHE[key] = _build(stage)
    return _CACHE[key]


def prep_weights(Wv, bv, Wq, bq, wa, Wo, bo):
    """Host-side layout prep of the replicated weights (pure layout/dtype)."""
    f16, f32 = np.float16, np.float32
    Wq16 = np.asarray(Wq, f32).astype(f16)
    Wv16 = np.asarray(Wv, f32).astype(f16)
    Wo = np.asarray(Wo, f32)
    Wo16 = (
        np.transpose(Wo.reshape(4, 128, H_DIM), (1, 0, 2))
        .reshape(128, 4 * H_DIM)
        .astype(f16)
    )
    bq16 = np.asarray(bq, f32).reshape(1, HD).astype(f16)
    bqb16 = np.broadcast_to(bq16, (128, HD))
    bv_T = np.asarray(bv, f32).reshape(4, 128).T            # [128, 4]
    bq_T = np.asarray(bq, f32).reshape(4, 128).T            # [128, 4]
    wa_T = np.asarray(wa, f32).reshape(2, 128).T            # [128, 2]
    wa_T4 = np.concatenate([wa_T, wa_T], axis=1)            # [128, 4]
    bo_T = np.asarray(bo, f32).reshape(2, 128).T            # [128, 2]
    CT32 = np.ascontiguousarray(
        np.concatenate([bv_T, bq_T * wa_T4, wa_T4, bo_T], axis=1).astype(f32)
    )
    return {"CT32": CT32}, Wq16, Wv16, Wo16, bqb16


def prep_example(Wq16, Wv16, Wo16, bqb16, v_b, q_b):
    """Pack per-core fp16 operands (weights replicated + this example's v/q)."""
    f16 = np.float16
    vT16 = np.ascontiguousarray(np.asarray(v_b, np.float32).T).astype(f16)
    qT16 = np.ascontiguousarray(np.asarray(q_b, np.float32).T).astype(f16)
    pk = np.concatenate([Wv16, vT16, Wq16, qT16, Wo16, bqb16], axis=1)
    assert pk.shape == (128, PK_COLS)
    return {"PK16": np.ascontiguousarray(pk)}


def unprep_att(attT16):
    """[128 j, 8 (h,jc), 128 i] fp16 -> [2, 128 i, 512 j] fp32."""
    a = np.asarray(attT16).reshape(128, 2, 4, 128).astype(np.float32)
    return np.transpose(a, (1, 3, 2, 0)).reshape(H_OUT, NV, NQ)


def make_in_maps(v, q, Wv, bv, Wq, bq, wa, Wo, bo):
    common, Wq16, Wv16, Wo16, bqb16 = prep_weights(Wv, bv, Wq, bq, wa, Wo, bo)
    B = np.asarray(v).shape[0]
    return [
        dict(common, **prep_example(Wq16, Wv16, Wo16, bqb16, v[b], q[b]))
        for b in range(B)
    ]


def kernel(v, q, Wv, bv, Wq, bq, wa, ba, Wo, bo):
    from concourse.bass_utils import run_bass_kernel_spmd

    nc = get_nc()
    B = np.asarray(v).shape[0]
    assert B == N_CORES, f"expected batch {N_CORES}, got {B}"
    in_maps = make_in_maps(v, q, Wv, bv, Wq, bq, wa, Wo, bo)
    res = run_bass_kernel_spmd(nc, in_maps, core_ids=list(range(N_CORES)))
    fused = np.stack(
        [r["fusedT"].T.reshape(H_DIM) for r in res.results], axis=0
    )
    att = np.stack([unprep_att(r["attT16"]) for r in res.results], axis=0)
    att = att.reshape(B, H_OUT, NV * NQ, 1)
    return fused, att
